# revision 3
# baseline (speedup 1.0000x reference)
"""Trainium2 Bass kernel for DCTLAVISBlip dc_transform (DCT -> truncate -> IDCT).

v3 = v2 (even/odd DCT fold, see below) + three trace-driven fixes:
  - uniform matmul tile_size (128,128): the K=32 contraction remainder is
    packed for each batch-pair as one fully-initialized [128,1024] SBUF
    tile holding the four (batch,side) 32-row remainders, multiplied by
    host-built weight tiles that are zero outside the matching 32-row
    band. v2's K=32 instructions forced a PE tile reconfig that blocked
    stationary-weight prefetch (~83 matmuls at ~320ns instead of ~215).
  - PE warmup moved off the critical path: the zero tile is cleared by
    the scalar engine (starts ~0.2us) instead of gpsimd (~6us), so the
    36 ramp matmuls finish before the first input tiles land.
  - output DMA is triggered by the same engine that drained the tile
    (vector/scalar), so the final tiles' trigger latency overlaps across
    queues instead of serializing on sync at the end.

Fold math: M[k, 575-t] = (-1)^k M[k, t], so with e = x[:288] + x_rev,
o = x[:288] - x_rev the even/odd DCT rows are two K=288 contractions
(y_even = M_e e, y_odd = M_o o), and the IDCT output symmetry
state[t] = a_t + b_t, state[L-1-t] = a_t - b_t gives a = P_a e,
b = P_b o. Device computes ye = [M_e[:ne]; P_a] e and
yo = [M_o[:no]; P_b] o per batch; host folds inputs and unfolds
outputs. 60 matmuls per batch (2 sides x 3 k x 5 m x 2 n-chunks) at
~215ns each; measured matmul cost is N columns regardless of K/M.
"""

import numpy as np

B, T, C = 64, 576, 1024
H = T // 2                   # fold length 288
NCORES = 8
BPC = B // NCORES            # batches per core
Q = 0.8

KF = [(0, 128), (128, 128)]  # full k-tiles; k=256:288 handled by rem pack
N_TILES = [(0, 512), (512, 512)]

_CACHED = {}


def _dct_mat(N):
    n = np.arange(N)
    Mm = np.cos(np.pi * (2 * n[None, :] + 1) * n[:, None] / (2 * N))
    s = np.full(N, np.sqrt(2.0 / N))
    s[0] = np.sqrt(1.0 / N)
    return s[:, None] * Mm          # float64


def _m_tiles(rows):
    out = []
    m0 = 0
    while m0 < rows:
        out.append((m0, min(128, rows - m0)))
        m0 += 128
    return out


def _build_nc(mt_e, mt_o):
    import concourse.bacc as bacc
    import concourse.mybir as mybir
    import concourse.tile as tile

    f16 = mybir.dt.float16
    f32 = mybir.dt.float32
    mtmax = max(mt_e, mt_o)

    nc = bacc.Bacc("TRN2", target_bir_lowering=False, debug=False,
                   num_devices=NCORES)
    eo = nc.dram_tensor("eo", [BPC, 2, 256, C], f16, kind="ExternalInput")
    reo = nc.dram_tensor("reo", [BPC // 2, 128, C], f16,
                         kind="ExternalInput")
    wte = nc.dram_tensor("wte", [256, mt_e], f16, kind="ExternalInput")
    wto = nc.dram_tensor("wto", [256, mt_o], f16, kind="ExternalInput")
    # remainder weights: quadrant q covers (batch-parity, side) with the
    # 32-row band [32q, 32q+32) holding W.T[256:288] and zeros elsewhere
    wr = nc.dram_tensor("wr", [128, 4 * mtmax], f16, kind="ExternalInput")
    ye = nc.dram_tensor("ye", [BPC, mt_e, C], f16, kind="ExternalOutput")
    yo = nc.dram_tensor("yo", [BPC, mt_o, C], f16, kind="ExternalOutput")

    wdr = (wte, wto)
    odr = (ye, yo)
    mts = (_m_tiles(mt_e), _m_tiles(mt_o))

    with tile.TileContext(nc) as tc:
        with (
            tc.tile_pool(name="wpool", bufs=1) as wpool,
            tc.tile_pool(name="xpool", bufs=1) as xpool,
            tc.tile_pool(name="osb", bufs=10) as opool,
            tc.tile_pool(name="ps", bufs=8, space="PSUM") as ps,
        ):
            # PE warmup: scalar clears the dummy weights almost immediately
            # after kernel start, so the ramp matmuls run during the
            # input-DMA head instead of delaying the first real matmul.
            wz = wpool.tile([128, 128], f16, tag="wz")
            nc.scalar.memzero(wz[:])
            pwarm = ps.tile([128, 128], f32, tag="pt", name="pt")
            for _ in range(24):
                nc.tensor.matmul(pwarm[:], wz[:], wz[:], start=True, stop=True)

            # Input DMAs in first-use order (group 0: side e, then rem
            # pack, then side o; later pairs follow).
            wts = {}    # (s, ki) -> full-k weight tile
            xts = {}    # (b, s, ki) -> input tile
            rem = {}    # pair -> packed [128, C] remainder tile
            order0 = [("w", 0, 0), ("x", 0, 0, 0), ("x", 1, 0, 0),
                      ("w", 0, 1), ("x", 0, 0, 1), ("x", 1, 0, 1)]
            for item in order0:
                if item[0] == "w":
                    _, s, ki = item
                    k0, kk = KF[ki]
                    t_ = wpool.tile([kk, mts[s][-1][0] + mts[s][-1][1]],
                                    f16, tag=f"w{s}_{ki}", name=f"w{s}_{ki}")
                    nc.sync.dma_start(t_[:], wdr[s][k0:k0 + kk, :])
                    wts[(s, ki)] = t_
                else:
                    _, b, s, ki = item
                    k0, kk = KF[ki]
                    tx = xpool.tile([kk, C], f16, tag=f"x{b}_{s}_{ki}",
                                    name=f"x{b}_{s}_{ki}")
                    nc.sync.dma_start(tx[:], eo[b, s, k0:k0 + kk, :])
                    xts[(b, s, ki)] = tx
            wrt = wpool.tile([128, 4 * mtmax], f16, tag="wrt", name="wrt")
            nc.sync.dma_start(wrt[:], wr[:, :])

            def load_rem(gi):
                tr = xpool.tile([128, C], f16, tag=f"rem{gi}",
                                name=f"rem{gi}")
                nc.sync.dma_start(tr[:], reo[gi, :, :])
                rem[gi] = tr

            load_rem(0)
            for s, ki in (((1, 0)), (1, 1)):
                k0, kk = KF[ki]
                t_ = wpool.tile([kk, mts[s][-1][0] + mts[s][-1][1]],
                                f16, tag=f"w{s}_{ki}", name=f"w{s}_{ki}")
                nc.sync.dma_start(t_[:], wdr[s][k0:k0 + kk, :])
                wts[(s, ki)] = t_
                for b in (0, 1):
                    tx = xpool.tile([kk, C], f16, tag=f"x{b}_{s}_{ki}",
                                    name=f"x{b}_{s}_{ki}")
                    nc.sync.dma_start(tx[:], eo[b, s, k0:k0 + kk, :])
                    xts[(b, s, ki)] = tx
            for b in range(2, BPC):
                for s in (0, 1):
                    for ki, (k0, kk) in enumerate(KF):
                        tx = xpool.tile([kk, C], f16, tag=f"x{b}_{s}_{ki}",
                                        name=f"x{b}_{s}_{ki}")
                        nc.sync.dma_start(tx[:], eo[b, s, k0:k0 + kk, :])
                        xts[(b, s, ki)] = tx
                if b % 2 == 0:
                    load_rem(b // 2)

            for gi in range(BPC // 2):
                b0 = 2 * gi
                pairs = [(b0, 0), (b0, 1), (b0 + 1, 0), (b0 + 1, 1)]
                for s in (0, 1):
                    for mi, (m0, mm) in enumerate(mts[s]):
                        pts = []
                        for _ in pairs:
                            pts.append(ps.tile([128, 512], f32, tag="pt",
                                               name="pt"))
                        for ki in range(2):
                            for pi, (b, n) in enumerate(pairs):
                                n0, nn = N_TILES[n]
                                nc.tensor.matmul(
                                    pts[pi][0:mm, :],
                                    wts[(s, ki)][:, m0:m0 + mm],
                                    xts[(b, s, ki)][:, n0:n0 + nn],
                                    start=(ki == 0),
                                    stop=False,
                                )
                        for pi, (b, n) in enumerate(pairs):
                            n0, nn = N_TILES[n]
                            q = 2 * (b - b0) + s
                            nc.tensor.matmul(
                                pts[pi][0:mm, :],
                                wrt[:, q * mtmax + m0:q * mtmax + m0 + mm],
                                rem[gi][:, n0:n0 + nn],
                                start=False,
                                stop=True,
                            )
                        # drain psum -> sbuf f16 -> dram. Trigger engines are
                        # kept OFF the drain engines (a DIRECT2D trigger on
                        # scalar/vector delays the drains -> psum
                        # backpressure -> PE stalls). Early groups trigger on
                        # gpsimd (sync is busy issuing input DMAs); later
                        # groups alternate sync/gpsimd so the final tiles'
                        # trigger+doorbell latency overlaps across queues.
                        for bi, b in enumerate((b0, b0 + 1)):
                            p0, p1 = 2 * bi, 2 * bi + 1
                            ot = opool.tile([128, C], f16, tag="ot")
                            # both engines drain each tile (one n-half
                            # each): the psum quad frees ~0.65us sooner,
                            # shrinking the matmul stalls at quad
                            # boundaries
                            nc.vector.tensor_copy(ot[0:mm, 0:512],
                                                  pts[p0][0:mm, :])
                            nc.scalar.copy(ot[0:mm, 512:1024],
                                           pts[p1][0:mm, :])
                            if gi < 2:
                                oeng = nc.gpsimd
                            elif gi == 3 and s == 1:
                                # kernel end: avoid gpsimd (its end-of-queue
                                # drain costs ~8us after its last DMA, so
                                # its last trigger must come early); scalar
                                # fires right after its own copy
                                if bi == 1 and mi >= 3:
                                    oeng = nc.scalar
                                else:
                                    oeng = nc.sync
                            else:
                                oeng = (nc.sync, nc.gpsimd)[(s * 5 + mi + bi) % 2]
                            oeng.dma_start(
                                odr[s][b, m0:m0 + mm, :], ot[0:mm, :])
    nc.finalize()
    return nc


def _get_nc(mt_e, mt_o):
    key = ("nc", mt_e, mt_o)
    if key not in _CACHED:
        _CACHED[key] = _build_nc(mt_e, mt_o)
    return _CACHED[key]


def _ensure_trace_hook_safe():
    """If BASS_TRACE is set, run_bass_kernel_spmd imports antenv.axon_hooks,
    which may not exist. Install a ctypes-based shim when possible, else
    disable tracing so the run cannot crash."""
    import os
    import sys
    import types

    if not os.environ.get("BASS_TRACE"):
        return
    try:
        import antenv.axon_hooks  # noqa: F401
        return
    except ImportError:
        pass
    try:
        from trn_agent_boot.trn_boot import _ntff_profile_via_ctypes
        hooks = types.ModuleType("antenv.axon_hooks")
        hook = _ntff_profile_via_ctypes("/opt/axon/libaxon_pjrt.so")
        hooks.get_axon_ntff_profile_hook = lambda: hook
        hooks.set_axon_ntff_profile_hook = lambda h: None
        sys.modules["antenv.axon_hooks"] = hooks
    except Exception:
        os.environ["BASS_NEVER_TRACE"] = "1"


def kernel(x: np.ndarray):
    from concourse.bass_utils import run_bass_kernel_spmd

    _ensure_trace_hook_safe()
    x = np.ascontiguousarray(np.asarray(x, dtype=np.float32))
    assert x.shape == (B, T, C)

    # ---- host: data-dependent truncation length L (tiny, exact math) ----
    M64 = _dct_mat(T)
    xbar = x.astype(np.float64).mean(axis=(0, 2))
    v = np.abs(M64 @ xbar)
    thr = np.abs(np.quantile(v, Q))
    idxs = np.where(v > thr)[0]
    last_index = int(idxs[-1]) if idxs.size > 0 else -1
    L = last_index if last_index >= 0 else T - 1

    ne = (L + 1) // 2        # even y rows kept
    no = L // 2              # odd y rows kept
    cl2 = (L + 1) // 2       # first-half state rows
    mt_e = ne + cl2
    mt_o = no + cl2
    mtmax = max(mt_e, mt_o)

    # ---- host: folded inputs and stacked fold-domain weights ----
    Me = M64[0::2, :H]                       # [288, 288]
    Mo = M64[1::2, :H]
    Mi = _dct_mat(L)
    Pa = Mi[0::2, :cl2].T @ Me[:ne, :]       # [cl2, 288]
    Pb = Mi[1::2, :cl2].T @ Mo[:no, :]
    We = np.concatenate([Me[:ne], Pa], axis=0)     # [mt_e, 288]
    Wo = np.concatenate([Mo[:no], Pb], axis=0)     # [mt_o, 288]
    WeT = np.ascontiguousarray(We.T).astype(np.float16)   # [288, mt_e]
    WoT = np.ascontiguousarray(Wo.T).astype(np.float16)
    wte16 = np.ascontiguousarray(WeT[:256])
    wto16 = np.ascontiguousarray(WoT[:256])
    wr16 = np.zeros((128, 4 * mtmax), dtype=np.float16)
    wr16[0:32, 0 * mtmax:0 * mtmax + mt_e] = WeT[256:H]
    wr16[32:64, 1 * mtmax:1 * mtmax + mt_o] = WoT[256:H]
    wr16[64:96, 2 * mtmax:2 * mtmax + mt_e] = WeT[256:H]
    wr16[96:128, 3 * mtmax:3 * mtmax + mt_o] = WoT[256:H]

    xlo = x[:, :H, :]
    xhi = x[:, T - 1:H - 1:-1, :]
    eof = np.empty((B, 2, H, C), dtype=np.float16)
    eof[:, 0] = xlo + xhi
    eof[:, 1] = xlo - xhi
    eo = eof[:, :, :256]
    reo = np.empty((B // 2, 128, C), dtype=np.float16)
    reo[:, 0:32] = eof[0::2, 0, 256:H]
    reo[:, 32:64] = eof[0::2, 1, 256:H]
    reo[:, 64:96] = eof[1::2, 0, 256:H]
    reo[:, 96:128] = eof[1::2, 1, 256:H]

    nc = _get_nc(mt_e, mt_o)
    gpc = BPC // 2
    in_maps = [
        {"eo": np.ascontiguousarray(eo[i * BPC:(i + 1) * BPC]),
         "reo": np.ascontiguousarray(reo[i * gpc:(i + 1) * gpc]),
         "wte": wte16, "wto": wto16, "wr": wr16}
        for i in range(NCORES)
    ]
    res = run_bass_kernel_spmd(nc, in_maps, list(range(NCORES)))
    _CACHED["last_exec_time_ns"] = res.exec_time_ns

    ye = np.concatenate([res.results[i]["ye"] for i in range(NCORES)], axis=0)
    yo = np.concatenate([res.results[i]["yo"] for i in range(NCORES)], axis=0)

    # ---- host: unfold outputs ----
    y = np.empty((B, L, C), dtype=np.float32)
    y[:, 0::2] = ye[:, :ne].astype(np.float32)
    y[:, 1::2] = yo[:, :no].astype(np.float32)
    a = ye[:, ne:].astype(np.float32)        # [B, cl2, C]
    bb = yo[:, no:].astype(np.float32)       # [B, cl2, C]
    st = np.empty((B, L, C), dtype=np.float32)
    st[:, :cl2] = a + bb
    st[:, cl2:] = (a - bb)[:, :L // 2][:, ::-1]
    state = np.ascontiguousarray(st.astype(np.float16))
    return state, y


# revision 5
# speedup vs baseline: 1.0396x; 1.0396x over previous
"""Trainium2 Bass kernel for DCTLAVISBlip dc_transform (DCT -> truncate -> IDCT).

Algorithm: even/odd DCT fold
----------------------------
reference(x), x [B=64, T=576, C=1024] f32, computes y = DCT_II(x) over
tokens, a data-dependent truncation y[:, :L, :], and state = IDCT_L of
the truncated rows (f16). The DCT matrix obeys M[k, T-1-t] = (-1)^k
M[k, t], so with host-folded inputs e = x[:288] + x_rev and
o = x[:288] - x_rev the even/odd DCT rows become two independent K=288
contractions (y_even = M_e e, y_odd = M_o o), and the IDCT's output
symmetry state[t] = a_t + b_t, state[L-1-t] = a_t - b_t gives a = P_a e
and b = P_b o with host-precomputed P_a = Mi_even.T M_e[:ne],
P_b = Mi_odd.T M_o[:no]. The device runs two stacked f16 matmuls per
batch, ye = [M_e[:ne]; P_a] e and yo = [M_o[:no]; P_b] o (each
[574, 288] @ [288, 1024] for this input's L=574), data-parallel over
batch across 8 cores (8 batches each); the host computes L (the tiny
quantile threshold), folds inputs, and unfolds/interleaves outputs.
This nearly halves PE work vs the unfolded [1152, 576] formulation.

Hardware choreography (trace-driven, see git of /tmp traces):
  - a matmul instruction costs its N output columns (~215 ns for N=512
    f16 at ~2.5 GHz) regardless of K or M, and tile_position-packed
    matmuls do NOT overlap; so the kernel minimizes instruction count:
    60 matmuls per batch = 2 sides x 3 k-tiles x 5 m-tiles x 2 n-chunks.
  - uniform tile_size (128,128): the K=32 contraction remainder is
    packed per batch-pair as one fully-initialized [128,1024] tile (the
    four (batch,side) 32-row remainders, host-prepacked in `reo`),
    against host-built weight tiles zero outside the matching band
    (`wr`). K=32 instructions would force a PE tile reconfig that
    blocks stationary-weight prefetch (+~115 ns each).
  - 4-wide PSUM quads (two batches x two 512-col chunks) reuse the
    stationary weight; 8 psum banks give two-quad double buffering.
  - both drain engines (vector + scalar) share every tile (one n-half
    each) so psum quads free sooner; drains never issue DMA triggers
    (a DIRECT2D on a drain engine delays drains -> psum backpressure
    -> PE stalls).
  - output DMA triggers: gpsimd for early groups (sync is issuing input
    DMAs, ~0.6us serial per trigger), sync/gpsimd alternation later;
    the final group's last tiles avoid gpsimd entirely because its
    end-of-queue drain costs ~10us after its last DMA (it must hide
    under compute), using sync + scalar (scalar fires right after its
    own copies) instead.
  - remainder weights/inputs ship as single packed tensors (one DMA
    trigger each) so the first k2 matmul isn't trigger-starved.
  - 24 dummy matmuls ramp the PE p-state during the input-DMA head.

Measured ~130.5-131.6 us on hardware (baseline before this session:
~170-177 us), rel err ~7e-4 vs the 2e-2 gate (f16 quantization).
"""

import numpy as np

B, T, C = 64, 576, 1024
H = T // 2                   # fold length 288
NCORES = 8
BPC = B // NCORES            # batches per core
Q = 0.8

KF = [(0, 128), (128, 128)]  # full k-tiles; k=256:288 handled by rem pack
N_TILES = [(0, 512), (512, 512)]

_CACHED = {}


def _dct_mat(N):
    n = np.arange(N)
    Mm = np.cos(np.pi * (2 * n[None, :] + 1) * n[:, None] / (2 * N))
    s = np.full(N, np.sqrt(2.0 / N))
    s[0] = np.sqrt(1.0 / N)
    return s[:, None] * Mm          # float64


def _m_tiles(rows):
    out = []
    m0 = 0
    while m0 < rows:
        out.append((m0, min(128, rows - m0)))
        m0 += 128
    return out


def _build_nc(mt_e, mt_o):
    import concourse.bacc as bacc
    import concourse.mybir as mybir
    import concourse.tile as tile

    f16 = mybir.dt.float16
    f32 = mybir.dt.float32
    mtmax = max(mt_e, mt_o)

    nc = bacc.Bacc("TRN2", target_bir_lowering=False, debug=False,
                   num_devices=NCORES)
    eo = nc.dram_tensor("eo", [BPC, 2, 256, C], f16, kind="ExternalInput")
    reo = nc.dram_tensor("reo", [BPC // 2, 128, C], f16,
                         kind="ExternalInput")
    wte = nc.dram_tensor("wte", [256, mt_e], f16, kind="ExternalInput")
    wto = nc.dram_tensor("wto", [256, mt_o], f16, kind="ExternalInput")
    # remainder weights: quadrant q covers (batch-parity, side) with the
    # 32-row band [32q, 32q+32) holding W.T[256:288] and zeros elsewhere
    wr = nc.dram_tensor("wr", [128, 4 * mtmax], f16, kind="ExternalInput")
    ye = nc.dram_tensor("ye", [BPC, mt_e, C], f16, kind="ExternalOutput")
    yo = nc.dram_tensor("yo", [BPC, mt_o, C], f16, kind="ExternalOutput")

    wdr = (wte, wto)
    odr = (ye, yo)
    mts = (_m_tiles(mt_e), _m_tiles(mt_o))

    with tile.TileContext(nc) as tc:
        with (
            tc.tile_pool(name="wpool", bufs=1) as wpool,
            tc.tile_pool(name="xpool", bufs=1) as xpool,
            tc.tile_pool(name="osb", bufs=10) as opool,
            tc.tile_pool(name="ps", bufs=8, space="PSUM") as ps,
        ):
            # PE warmup: scalar clears the dummy weights almost immediately
            # after kernel start, so the ramp matmuls run during the
            # input-DMA head instead of delaying the first real matmul.
            wz = wpool.tile([128, 128], f16, tag="wz")
            nc.scalar.memzero(wz[:])
            pwarm = ps.tile([128, 128], f32, tag="pt", name="pt")
            for _ in range(24):
                nc.tensor.matmul(pwarm[:], wz[:], wz[:], start=True, stop=True)

            # Input DMAs in first-use order (group 0: side e, then rem
            # pack, then side o; later pairs follow).
            wts = {}    # (s, ki) -> full-k weight tile
            xts = {}    # (b, s, ki) -> input tile
            rem = {}    # pair -> packed [128, C] remainder tile
            order0 = [("w", 0, 0), ("x", 0, 0, 0), ("x", 1, 0, 0),
                      ("w", 0, 1), ("x", 0, 0, 1), ("x", 1, 0, 1)]
            for item in order0:
                if item[0] == "w":
                    _, s, ki = item
                    k0, kk = KF[ki]
                    t_ = wpool.tile([kk, mts[s][-1][0] + mts[s][-1][1]],
                                    f16, tag=f"w{s}_{ki}", name=f"w{s}_{ki}")
                    nc.sync.dma_start(t_[:], wdr[s][k0:k0 + kk, :])
                    wts[(s, ki)] = t_
                else:
                    _, b, s, ki = item
                    k0, kk = KF[ki]
                    tx = xpool.tile([kk, C], f16, tag=f"x{b}_{s}_{ki}",
                                    name=f"x{b}_{s}_{ki}")
                    nc.sync.dma_start(tx[:], eo[b, s, k0:k0 + kk, :])
                    xts[(b, s, ki)] = tx
            wrt = wpool.tile([128, 4 * mtmax], f16, tag="wrt", name="wrt")
            nc.sync.dma_start(wrt[:], wr[:, :])

            def load_rem(gi):
                tr = xpool.tile([128, C], f16, tag=f"rem{gi}",
                                name=f"rem{gi}")
                nc.sync.dma_start(tr[:], reo[gi, :, :])
                rem[gi] = tr

            load_rem(0)
            for s, ki in (((1, 0)), (1, 1)):
                k0, kk = KF[ki]
                t_ = wpool.tile([kk, mts[s][-1][0] + mts[s][-1][1]],
                                f16, tag=f"w{s}_{ki}", name=f"w{s}_{ki}")
                nc.sync.dma_start(t_[:], wdr[s][k0:k0 + kk, :])
                wts[(s, ki)] = t_
                for b in (0, 1):
                    tx = xpool.tile([kk, C], f16, tag=f"x{b}_{s}_{ki}",
                                    name=f"x{b}_{s}_{ki}")
                    nc.sync.dma_start(tx[:], eo[b, s, k0:k0 + kk, :])
                    xts[(b, s, ki)] = tx
            for b in range(2, BPC):
                for s in (0, 1):
                    for ki, (k0, kk) in enumerate(KF):
                        tx = xpool.tile([kk, C], f16, tag=f"x{b}_{s}_{ki}",
                                        name=f"x{b}_{s}_{ki}")
                        nc.sync.dma_start(tx[:], eo[b, s, k0:k0 + kk, :])
                        xts[(b, s, ki)] = tx
                if b % 2 == 0:
                    load_rem(b // 2)

            for gi in range(BPC // 2):
                b0 = 2 * gi
                pairs = [(b0, 0), (b0, 1), (b0 + 1, 0), (b0 + 1, 1)]
                for s in (0, 1):
                    for mi, (m0, mm) in enumerate(mts[s]):
                        pts = []
                        for _ in pairs:
                            pts.append(ps.tile([128, 512], f32, tag="pt",
                                               name="pt"))
                        for ki in range(2):
                            for pi, (b, n) in enumerate(pairs):
                                n0, nn = N_TILES[n]
                                nc.tensor.matmul(
                                    pts[pi][0:mm, :],
                                    wts[(s, ki)][:, m0:m0 + mm],
                                    xts[(b, s, ki)][:, n0:n0 + nn],
                                    start=(ki == 0),
                                    stop=False,
                                )
                        for pi, (b, n) in enumerate(pairs):
                            n0, nn = N_TILES[n]
                            q = 2 * (b - b0) + s
                            nc.tensor.matmul(
                                pts[pi][0:mm, :],
                                wrt[:, q * mtmax + m0:q * mtmax + m0 + mm],
                                rem[gi][:, n0:n0 + nn],
                                start=False,
                                stop=True,
                            )
                        # drain psum -> sbuf f16 -> dram. Trigger engines are
                        # kept OFF the drain engines (a DIRECT2D trigger on
                        # scalar/vector delays the drains -> psum
                        # backpressure -> PE stalls). Early groups trigger on
                        # gpsimd (sync is busy issuing input DMAs); later
                        # groups alternate sync/gpsimd so the final tiles'
                        # trigger+doorbell latency overlaps across queues.
                        for bi, b in enumerate((b0, b0 + 1)):
                            p0, p1 = 2 * bi, 2 * bi + 1
                            ot = opool.tile([128, C], f16, tag="ot")
                            # both engines drain each tile (one n-half
                            # each): the psum quad frees ~0.65us sooner,
                            # shrinking the matmul stalls at quad
                            # boundaries
                            nc.vector.tensor_copy(ot[0:mm, 0:512],
                                                  pts[p0][0:mm, :])
                            nc.scalar.copy(ot[0:mm, 512:1024],
                                           pts[p1][0:mm, :])
                            if gi < 2:
                                oeng = nc.gpsimd
                            elif gi == 3 and s == 1:
                                # kernel end: avoid gpsimd (its end-of-queue
                                # drain costs ~8us after its last DMA, so
                                # its last trigger must come early); scalar
                                # fires right after its own copy
                                if bi == 1 and mi >= 3:
                                    oeng = nc.scalar
                                else:
                                    oeng = nc.sync
                            else:
                                oeng = (nc.sync, nc.gpsimd)[(s * 5 + mi + bi) % 2]
                            oeng.dma_start(
                                odr[s][b, m0:m0 + mm, :], ot[0:mm, :])
    nc.finalize()
    return nc


def _get_nc(mt_e, mt_o):
    key = ("nc", mt_e, mt_o)
    if key not in _CACHED:
        _CACHED[key] = _build_nc(mt_e, mt_o)
    return _CACHED[key]


def _ensure_trace_hook_safe():
    """If BASS_TRACE is set, run_bass_kernel_spmd imports antenv.axon_hooks,
    which may not exist. Install a ctypes-based shim when possible, else
    disable tracing so the run cannot crash."""
    import os
    import sys
    import types

    if not os.environ.get("BASS_TRACE"):
        return
    try:
        import antenv.axon_hooks  # noqa: F401
        return
    except ImportError:
        pass
    try:
        from trn_agent_boot.trn_boot import _ntff_profile_via_ctypes
        hooks = types.ModuleType("antenv.axon_hooks")
        hook = _ntff_profile_via_ctypes("/opt/axon/libaxon_pjrt.so")
        hooks.get_axon_ntff_profile_hook = lambda: hook
        hooks.set_axon_ntff_profile_hook = lambda h: None
        sys.modules["antenv.axon_hooks"] = hooks
    except Exception:
        os.environ["BASS_NEVER_TRACE"] = "1"


def kernel(x: np.ndarray):
    from concourse.bass_utils import run_bass_kernel_spmd

    _ensure_trace_hook_safe()
    x = np.ascontiguousarray(np.asarray(x, dtype=np.float32))
    assert x.shape == (B, T, C)

    # ---- host: data-dependent truncation length L (tiny, exact math) ----
    M64 = _dct_mat(T)
    xbar = x.astype(np.float64).mean(axis=(0, 2))
    v = np.abs(M64 @ xbar)
    thr = np.abs(np.quantile(v, Q))
    idxs = np.where(v > thr)[0]
    last_index = int(idxs[-1]) if idxs.size > 0 else -1
    L = last_index if last_index >= 0 else T - 1

    ne = (L + 1) // 2        # even y rows kept
    no = L // 2              # odd y rows kept
    cl2 = (L + 1) // 2       # first-half state rows
    mt_e = ne + cl2
    mt_o = no + cl2
    mtmax = max(mt_e, mt_o)

    # ---- host: folded inputs and stacked fold-domain weights ----
    Me = M64[0::2, :H]                       # [288, 288]
    Mo = M64[1::2, :H]
    Mi = _dct_mat(L)
    Pa = Mi[0::2, :cl2].T @ Me[:ne, :]       # [cl2, 288]
    Pb = Mi[1::2, :cl2].T @ Mo[:no, :]
    We = np.concatenate([Me[:ne], Pa], axis=0)     # [mt_e, 288]
    Wo = np.concatenate([Mo[:no], Pb], axis=0)     # [mt_o, 288]
    WeT = np.ascontiguousarray(We.T).astype(np.float16)   # [288, mt_e]
    WoT = np.ascontiguousarray(Wo.T).astype(np.float16)
    wte16 = np.ascontiguousarray(WeT[:256])
    wto16 = np.ascontiguousarray(WoT[:256])
    wr16 = np.zeros((128, 4 * mtmax), dtype=np.float16)
    wr16[0:32, 0 * mtmax:0 * mtmax + mt_e] = WeT[256:H]
    wr16[32:64, 1 * mtmax:1 * mtmax + mt_o] = WoT[256:H]
    wr16[64:96, 2 * mtmax:2 * mtmax + mt_e] = WeT[256:H]
    wr16[96:128, 3 * mtmax:3 * mtmax + mt_o] = WoT[256:H]

    xlo = x[:, :H, :]
    xhi = x[:, T - 1:H - 1:-1, :]
    eof = np.empty((B, 2, H, C), dtype=np.float16)
    eof[:, 0] = xlo + xhi
    eof[:, 1] = xlo - xhi
    eo = eof[:, :, :256]
    reo = np.empty((B // 2, 128, C), dtype=np.float16)
    reo[:, 0:32] = eof[0::2, 0, 256:H]
    reo[:, 32:64] = eof[0::2, 1, 256:H]
    reo[:, 64:96] = eof[1::2, 0, 256:H]
    reo[:, 96:128] = eof[1::2, 1, 256:H]

    nc = _get_nc(mt_e, mt_o)
    gpc = BPC // 2
    in_maps = [
        {"eo": np.ascontiguousarray(eo[i * BPC:(i + 1) * BPC]),
         "reo": np.ascontiguousarray(reo[i * gpc:(i + 1) * gpc]),
         "wte": wte16, "wto": wto16, "wr": wr16}
        for i in range(NCORES)
    ]
    res = run_bass_kernel_spmd(nc, in_maps, list(range(NCORES)))
    _CACHED["last_exec_time_ns"] = res.exec_time_ns

    ye = np.concatenate([res.results[i]["ye"] for i in range(NCORES)], axis=0)
    yo = np.concatenate([res.results[i]["yo"] for i in range(NCORES)], axis=0)

    # ---- host: unfold outputs ----
    y = np.empty((B, L, C), dtype=np.float32)
    y[:, 0::2] = ye[:, :ne].astype(np.float32)
    y[:, 1::2] = yo[:, :no].astype(np.float32)
    a = ye[:, ne:].astype(np.float32)        # [B, cl2, C]
    bb = yo[:, no:].astype(np.float32)       # [B, cl2, C]
    st = np.empty((B, L, C), dtype=np.float32)
    st[:, :cl2] = a + bb
    st[:, cl2:] = (a - bb)[:, :L // 2][:, ::-1]
    state = np.ascontiguousarray(st.astype(np.float16))
    return state, y
